# revision 25
# baseline (speedup 1.0000x reference)
"""Self-contained GAT kernel for 8 TRN2 NeuronCores (v2 — batched-DMA design).

kernel(**inputs) takes the FULL unsharded inputs (as produced by
setup_inputs) and returns the FULL [100000, 64] float32 output.

Design (vs v1):
- nodes dst-partitioned across 8 cores; edges dst-sorted into 128-dst
  windows, packed into 128-edge tile-columns (col-major slots).
- per-node table rows [h bf16 x64 | a_src f32 x8] = 160B; a_dst lives in a
  separate LOCAL per-dst table tA [SPAD+1, 8] f32.
- ONE batched indirect gather per 128 tile-columns (~16k rows) for the src
  rows, and one for the dst a_dst rows (eliminates 1.1us/tile SWDGE fixed
  cost: 994ns + 0.34ns/desc).
- NO DMA transposes, NO per-tile PE transposes: a_dst comes from the second
  gather; alpha_src1/alpha_dst1 are folded into the node-phase matmul
  (Wcat = [W1 | W1@a_src1 | W1@a_dst1]); alpha2 folds computed per window
  with DVE mult+reduce in node-major layout.
- per-edge weights w = h[src]*p built with single 4D-AP DVE ops per
  16-tile group; denominator column rides inside the U matmul rhs.
- layer-2 output computed transposed (outT [64, SPAD]) via 4 PE transposes
  of the normalized aggregate per window; host transposes back (free).
"""
import os
import sys
import types

import numpy as np

sys.path.insert(0, "/opt/trn_rl_repo")

import ml_dtypes

import concourse.bass as bass
import concourse.bacc as bacc
import concourse.mybir as mybir
import concourse.tile as tile

BF16 = mybir.dt.bfloat16
F32 = mybir.dt.float32
I32 = mybir.dt.int32
U16 = mybir.dt.uint16

P = 128
H = 8
F1 = 8
F2 = 64
D1 = H * F1          # 64
IN_DIM = 256
NEG = 0.2
GHOST_AS = -300.0
TCOL = 80            # u16 per table row: 64 bf16 h | 8 f32 a_src
NC = 8
N = 100000
GRP = 16             # tile-columns per DVE group

LAST_EXEC_NS = None

_hook_registered = [False]


def _register_profile_hook():
    if _hook_registered[0]:
        return
    try:
        import antenv
        mod = types.ModuleType("antenv.axon_hooks")
        _h = [None]
        mod.set_axon_ntff_profile_hook = lambda f: _h.__setitem__(0, f)
        mod.get_axon_ntff_profile_hook = lambda: _h[0]
        sys.modules.setdefault("antenv.axon_hooks", mod)
        if not hasattr(antenv, "axon_hooks"):
            antenv.axon_hooks = mod
        from trn_agent_boot.trn_boot import _ntff_profile_via_ctypes
        sys.modules["antenv.axon_hooks"].set_axon_ntff_profile_hook(
            _ntff_profile_via_ctypes('/opt/axon/libaxon_pjrt.so'))
        _hook_registered[0] = True
    except Exception:
        pass


def mid_bcast(ap2d, reps):
    return bass.AP(ap2d.tensor, ap2d.offset, [ap2d.ap[0], [0, reps], ap2d.ap[1]])


def h_bcast(ap3d, reps):
    # [p, t, f] -> [p, t, reps(bc), f]
    return bass.AP(ap3d.tensor, ap3d.offset,
                   [ap3d.ap[0], ap3d.ap[1], [0, reps], ap3d.ap[2]])


def host_prep(inputs):
    SLICE = N // NC
    NW = (SLICE + P - 1) // P
    SPAD = NW * P
    GHOST = NC * SPAD

    edge = np.asarray(inputs["edge"])
    src = np.concatenate([np.asarray(edge[0]), np.arange(N, dtype=np.int64)])
    dst = np.concatenate([np.asarray(edge[1]), np.arange(N, dtype=np.int64)])

    core = (dst // SLICE).astype(np.int32)
    srcpad = ((src // SLICE) * SPAD + (src % SLICE)).astype(np.int32)
    dstl = (dst % SLICE).astype(np.int32)
    win = dstl // P

    counts = np.zeros((NC, NW), np.int64)
    for c in range(NC):
        m = core == c
        w, cnt = np.unique(win[m], return_counts=True)
        counts[c, w] = cnt
    T_w = np.maximum(1, (counts.max(axis=0) + P - 1) // P).astype(np.int64)
    T_tot = int(T_w.sum())
    col0 = np.concatenate([[0], np.cumsum(T_w)[:-1]])

    srcoff = np.full((NC, P, T_tot), GHOST, np.int32)
    dstrel = np.zeros((NC, P, T_tot), np.float32)
    order = np.argsort(core * np.int64(SLICE * 2) + dstl, kind="stable")
    s_s, d_s, c_s, w_s = srcpad[order], dstl[order], core[order], win[order]
    for c in range(NC):
        m = c_s == c
        sc, dc, wc = s_s[m], d_s[m], w_s[m]
        for w in range(NW):
            mw = wc == w
            k = int(mw.sum())
            tw = int(T_w[w])
            sl = np.full(tw * P, GHOST, np.int32)
            rl = np.zeros(tw * P, np.float32)
            sl[:k] = sc[mw]
            rl[:k] = (dc[mw] - w * P).astype(np.float32)
            cw = int(col0[w])
            srcoff[c, :, cw:cw + tw] = sl.reshape(tw, P).T
            dstrel[c, :, cw:cw + tw] = rl.reshape(tw, P).T

    # transposed one-hot stream: stS[d, t*128+p] = (dstrel[c][p, t] == d), fp8
    stS = np.zeros((NC, P, T_tot * P), ml_dtypes.float8_e4m3)
    for c in range(NC):
        dr = dstrel[c].astype(np.int64)          # [P, T_tot]
        pp_, tt_ = np.meshgrid(np.arange(P), np.arange(T_tot), indexing="ij")
        stS[c][dr.reshape(-1), (tt_ * P + pp_).reshape(-1)] = 1.0

    grow = np.zeros(TCOL, np.uint16)
    grow[64:80] = np.full(8, GHOST_AS, np.float32).view(np.uint16)

    W1 = np.asarray(inputs["W1"], np.float32)
    a_src1 = np.asarray(inputs["a_src1"], np.float32)
    a_dst1 = np.asarray(inputs["a_dst1"], np.float32)
    b1 = np.asarray(inputs["b1"], np.float32)
    W2 = np.asarray(inputs["W2"], np.float32)
    a_src2 = np.asarray(inputs["a_src2"], np.float32)
    a_dst2 = np.asarray(inputs["a_dst2"], np.float32)
    b2 = np.asarray(inputs["b2"], np.float32)
    x = np.asarray(inputs["x"], np.float32)

    # Wcat = [W1 (64) | W1-folded a_src1 (8) | W1-folded a_dst1 (8)] : [256, 80]
    W1r = W1.reshape(IN_DIM, H, F1)
    Ws1 = np.einsum("ihf,hf->ih", W1r, a_src1)
    Wd1 = np.einsum("ihf,hf->ih", W1r, a_dst1)
    Wcat = np.concatenate([W1, Ws1, Wd1], axis=1).astype(ml_dtypes.bfloat16)

    # layer2 attention folds: As2f[f, h] = sum_g W2[f, h*64+g] * a_src2[h, g]
    W2r = W2.reshape(D1, H, F2)
    As2f = np.einsum("fhg,hg->fh", W2r, a_src2)
    Ad2f = np.einsum("fhg,hg->fh", W2r, a_dst2)
    # AsAd2rep[p, k*64+f] = (As2f|Ad2f)[f, k], replicated over partitions
    AA = np.concatenate([As2f, Ad2f], axis=1)            # [64, 16]
    AArep = np.broadcast_to(AA.T.reshape(1, 16 * 64), (P, 16 * 64))
    AArep = np.ascontiguousarray(AArep).astype(ml_dtypes.bfloat16)

    # W2cb rows (h,f) -> g, pre-divided by H (head mean)
    W2cb = (W2r.transpose(1, 0, 2).reshape(H * D1, F2) / H).astype(ml_dtypes.bfloat16)

    iotaC = np.broadcast_to(np.arange(P, dtype=np.float32), (P, P)).astype(ml_dtypes.bfloat16)

    shared = dict(
        Wcat_a=np.ascontiguousarray(Wcat[0:P]),
        Wcat_b=np.ascontiguousarray(Wcat[P:2 * P]),
        AArep=AArep,
        W2cb=W2cb,
        b1rep=np.broadcast_to(b1, (P, D1)).copy(),
        b2col=np.ascontiguousarray(b2.reshape(F2, 1)),
        iotaC=np.ascontiguousarray(iotaC),
        I128=np.eye(P, dtype=np.float32),
        ghostrow=grow.reshape(1, TCOL),
    )
    in_maps = []
    for c in range(NC):
        xs = np.zeros((SPAD, IN_DIM), np.float32)
        xs[:SLICE] = x[c * SLICE:(c + 1) * SLICE]
        m = dict(shared)
        m["xTb"] = np.ascontiguousarray(xs.T).astype(ml_dtypes.bfloat16)
        m["srcoff"] = np.ascontiguousarray(srcoff[c])
        m["stS"] = np.ascontiguousarray(stS[c])
        m["dstrel"] = np.ascontiguousarray(dstrel[c]).astype(ml_dtypes.bfloat16)
        in_maps.append(m)

    meta = dict(SLICE=SLICE, NW=NW, SPAD=SPAD, GHOST=GHOST,
                T_w=[int(t) for t in T_w], col0=[int(cc) for cc in col0],
                T_tot=T_tot, NC=NC)
    return in_maps, meta


def build(meta):
    SLICE, NW, SPAD, GHOST, T_tot = (meta["SLICE"], meta["NW"], meta["SPAD"],
                                     meta["GHOST"], meta["T_tot"])
    T_w, col0 = meta["T_w"], meta["col0"]

    nc = bacc.Bacc('TRN2', num_devices=NC)
    FP8 = mybir.dt.float8e4
    xTb_d = nc.dram_tensor("xTb", [IN_DIM, SPAD], BF16, kind="ExternalInput")
    srcoff_d = nc.dram_tensor("srcoff", [P, T_tot], I32, kind="ExternalInput")
    stS_d = nc.dram_tensor("stS", [P, T_tot * P], FP8, kind="ExternalInput")
    dstrel_d = nc.dram_tensor("dstrel", [P, T_tot], BF16, kind="ExternalInput")
    Wcat_a_d = nc.dram_tensor("Wcat_a", [P, 80], BF16, kind="ExternalInput")
    Wcat_b_d = nc.dram_tensor("Wcat_b", [P, 80], BF16, kind="ExternalInput")
    AArep_d = nc.dram_tensor("AArep", [P, 1024], BF16, kind="ExternalInput")
    W2cb_d = nc.dram_tensor("W2cb", [H * D1, F2], BF16, kind="ExternalInput")
    b1rep_d = nc.dram_tensor("b1rep", [P, D1], F32, kind="ExternalInput")
    b2col_d = nc.dram_tensor("b2col", [F2, 1], F32, kind="ExternalInput")
    iotaC_d = nc.dram_tensor("iotaC", [P, P], BF16, kind="ExternalInput")
    I128_d = nc.dram_tensor("I128", [P, P], F32, kind="ExternalInput")
    ghostrow_d = nc.dram_tensor("ghostrow", [1, TCOL], U16, kind="ExternalInput")
    outT_d = nc.dram_tensor("outT", [F2, SPAD], F32, kind="ExternalOutput")
    t1loc = nc.dram_tensor("t1loc", [SPAD, TCOL], U16)
    t1full = nc.dram_tensor("t1full", [NC * SPAD + 1, TCOL], U16,
                            addr_space="Shared")
    t2loc = nc.dram_tensor("t2loc", [SPAD, TCOL], U16)
    t2full = nc.dram_tensor("t2full", [NC * SPAD + 1, TCOL], U16,
                            addr_space="Shared")

    # col -> window id
    win_of = np.zeros(T_tot, np.int64)
    for w in range(NW):
        win_of[col0[w]:col0[w] + T_w[w]] = w

    with tile.TileContext(nc) as tc:
        with tc.tile_pool(name="consts", bufs=1) as cpool, \
             tc.tile_pool(name="pg1", bufs=3) as pg1, \
             tc.tile_pool(name="pst", bufs=3) as pst, \
             tc.tile_pool(name="peb", bufs=3) as peb, \
             tc.tile_pool(name="ps16", bufs=3) as ps16, \
             tc.tile_pool(name="pw", bufs=3) as pw, \
             tc.tile_pool(name="ptail", bufs=2) as ptail, \
             tc.tile_pool(name="pnode", bufs=2) as pnode, \
             tc.tile_pool(name="ppU", bufs=2, space="PSUM") as ppU, \
             tc.tile_pool(name="ppA", bufs=2, space="PSUM") as ppA, \
             tc.tile_pool(name="pp2", bufs=1, space="PSUM") as pp2:

            def cload(dram, shape, dtype, tag):
                t = cpool.tile(shape, dtype, tag=tag)
                nc.sync.dma_start(out=t[:], in_=dram[:, :])
                return t

            Wcat_a = cload(Wcat_a_d, [P, 80], BF16, "cWa")
            Wcat_b = cload(Wcat_b_d, [P, 80], BF16, "cWb")
            AArep = cload(AArep_d, [P, 1024], BF16, "cAA")
            b1rep = cload(b1rep_d, [P, D1], F32, "cb1")
            b2col = cload(b2col_d, [F2, 1], F32, "cb2")
            iotaC = cload(iotaC_d, [P, P], BF16, "ciota")
            I128 = cload(I128_d, [P, P], F32, "cI128")
            grow_sb = cload(ghostrow_d, [1, TCOL], U16, "cgrow")
            srcoff_sb = cload(srcoff_d, [P, T_tot], I32, "csrcoff")
            dstrel_sb = cload(dstrel_d, [P, T_tot], BF16, "cdstrel")
            ad1w = cpool.tile([P, NW * 16], FP8, tag="cad1w")
            ad2w = cpool.tile([P, NW * 16], FP8, tag="cad2w")
            w2cb_sb = []
            for i in range(4):
                t = cpool.tile([P, F2], BF16, tag=f"cW2cb{i}")
                nc.sync.dma_start(out=t[:], in_=W2cb_d[i * P:(i + 1) * P, :])
                w2cb_sb.append(t)

            # ghost rows
            nc.sync.dma_start(out=t1full[GHOST:GHOST + 1, :], in_=grow_sb[:])
            nc.sync.dma_start(out=t2full[GHOST:GHOST + 1, :], in_=grow_sb[:])

            # ---------------- node phase ----------------
            CH = 512
            for k in range((SPAD + CH - 1) // CH):
                c0, c1 = k * CH, min((k + 1) * CH, SPAD)
                cw = c1 - c0
                xa = pnode.tile([P, CH], BF16, tag="xa")
                xb = pnode.tile([P, CH], BF16, tag="xb")
                nc.sync.dma_start(out=xa[:, :cw], in_=xTb_d[0:P, c0:c1])
                nc.sync.dma_start(out=xb[:, :cw], in_=xTb_d[P:2 * P, c0:c1])
                for b in range(cw // P):
                    n0 = c0 + b * P
                    h_full = pp2.tile([P, P], F32, space="PSUM", tag="Tps")
                    h_ps = h_full[:, 0:80]
                    nc.tensor.matmul(out=h_ps, lhsT=xa[:, b * P:(b + 1) * P],
                                     rhs=Wcat_a[:], start=True, stop=False)
                    nc.tensor.matmul(out=h_ps, lhsT=xb[:, b * P:(b + 1) * P],
                                     rhs=Wcat_b[:], start=False, stop=True)
                    hb = pnode.tile([P, D1], BF16, tag="hb")
                    nc.vector.tensor_copy(out=hb[:], in_=h_full[:, 0:D1])
                    asf = pnode.tile([P, 8], F32, tag="asf")
                    nc.vector.tensor_copy(out=asf[:], in_=h_full[:, 64:72])
                    wn = n0 // P
                    nc.vector.tensor_copy(out=ad1w[:, wn * 16:wn * 16 + 8],
                                          in_=h_full[:, 72:80])
                    nc.vector.tensor_tensor(out=ad1w[:, wn * 16 + 8:wn * 16 + 16],
                                            in0=h_full[:, 72:80],
                                            in1=ad1w[:, wn * 16:wn * 16 + 8],
                                            op=mybir.AluOpType.subtract)
                    nc.sync.dma_start(out=t1loc[n0:n0 + P, 0:D1].bitcast(BF16),
                                      in_=hb[:])
                    nc.sync.dma_start(out=t1loc[n0:n0 + P, D1:TCOL].bitcast(F32),
                                      in_=asf[:])

            nc.gpsimd.collective_compute(
                "AllGather", mybir.AluOpType.bypass,
                replica_groups=[list(range(NC))],
                ins=[t1loc[:, :].opt()],
                outs=[t1full[0:NC * SPAD, :].opt()],
            )

            # ---------------- edge phase ----------------
            def edge_phase(tfull, adw, layer):
                WC = 528 if layer == 2 else 72   # w row width (incl p slot+pad)
                POFF = 256 if layer == 2 else 64  # p slot offset
                w_all = [None] * ((T_tot + GRP - 1) // GRP)
                s_all = [None] * ((T_tot + GRP - 1) // GRP)

                def do_group(gi):
                    g0 = gi * GRP
                    gc = min(GRP, T_tot - g0)
                    g1 = pg1.tile([P, GRP * TCOL], U16, tag="g1")
                    for sl in range(gc):
                        t = g0 + sl
                        nc.gpsimd.indirect_dma_start(
                            out=g1[:, sl * TCOL:(sl + 1) * TCOL],
                            out_offset=None, in_=tfull[:, :],
                            in_offset=bass.IndirectOffsetOnAxis(
                                ap=srcoff_sb[:, t:t + 1], axis=0),
                        )
                    st16 = pst.tile([P, GRP * P], FP8, tag="st16")
                    nc.sync.dma_start(out=st16[:, :gc * P],
                                      in_=stS_d[:, g0 * P:(g0 + gc) * P])
                    adg = ppA.tile([P, GRP * 8], F32, space="PSUM", tag="adg")
                    for sl in range(gc):
                        wn = win_of[g0 + sl]
                        nc.tensor.matmul(out=adg[:, sl * 8:(sl + 1) * 8],
                                         lhsT=st16[:, sl * P:(sl + 1) * P],
                                         rhs=adw[:, wn * 16:wn * 16 + 8],
                                         start=True, stop=False)
                        nc.tensor.matmul(out=adg[:, sl * 8:(sl + 1) * 8],
                                         lhsT=st16[:, sl * P:(sl + 1) * P],
                                         rhs=adw[:, wn * 16 + 8:wn * 16 + 16],
                                         start=False, stop=True)
                    g1v = g1[:].rearrange("p (t c) -> p t c", c=TCOL)
                    e_b = peb.tile([P, GRP * 8], F32, tag="eb")
                    nc.vector.tensor_tensor(
                        out=e_b[:, :gc * 8].rearrange("p (t h) -> p t h", h=8),
                        in0=g1v[:, :gc, 64:80].bitcast(F32),
                        in1=adg[:, :gc * 8].rearrange("p (t h) -> p t h", h=8),
                        op=mybir.AluOpType.add)
                    lr_b = peb.tile([P, GRP * 8], F32, tag="lrb")
                    nc.vector.tensor_scalar_mul(out=lr_b[:, :gc * 8],
                                                in0=e_b[:, :gc * 8], scalar1=NEG)
                    nc.vector.tensor_tensor(out=lr_b[:, :gc * 8],
                                            in0=lr_b[:, :gc * 8],
                                            in1=e_b[:, :gc * 8],
                                            op=mybir.AluOpType.max)
                    s16 = ps16.tile([P, GRP * P], BF16, tag="s16")
                    nc.vector.tensor_tensor(
                        out=s16[:, :gc * P].rearrange("p (t c) -> p t c", c=P),
                        in0=dstrel_sb[:, g0:g0 + gc].to_broadcast([P, gc, P]),
                        in1=mid_bcast(iotaC[:], gc),
                        op=mybir.AluOpType.is_equal)
                    w16 = pw.tile([P, GRP * WC], BF16, tag=f"w{layer}")
                    wv = w16[:].rearrange("p (t c) -> p t c", c=WC)
                    # exp(lr) -> p slot (bf16)
                    nc.scalar.activation(
                        wv[:, :gc, POFF:POFF + 8],
                        lr_b[:].rearrange("p (t h) -> p t h", h=8)[:, 0:gc, :],
                        mybir.ActivationFunctionType.Exp)
                    hsl = g1v[:, 0:gc, 0:64].bitcast(BF16)
                    if layer == 1:
                        nc.vector.tensor_tensor(
                            out=wv[:, :gc, 0:64].rearrange("p t (h f) -> p t h f", h=8),
                            in0=hsl.rearrange("p t (h f) -> p t h f", h=8),
                            in1=wv[:, :gc, 64:72].to_broadcast([P, gc, 8, 8]),
                            op=mybir.AluOpType.mult)
                    else:
                        nc.vector.tensor_tensor(
                            out=wv[:, :gc, 0:256].rearrange("p t (h f) -> p t h f", h=4),
                            in0=h_bcast(hsl, 4),
                            in1=wv[:, :gc, 256:260].to_broadcast([P, gc, 4, 64]),
                            op=mybir.AluOpType.mult)
                        nc.vector.tensor_tensor(
                            out=wv[:, :gc, 264:520].rearrange("p t (h f) -> p t h f", h=4),
                            in0=h_bcast(hsl, 4),
                            in1=wv[:, :gc, 260:264].to_broadcast([P, gc, 4, 64]),
                            op=mybir.AluOpType.mult)
                    s_all[gi] = s16
                    w_all[gi] = w16

                for w in range(NW):
                    tw = T_w[w]
                    cw0 = col0[w]
                    if layer == 2:
                        Ua = ppU.tile([P, 264], F32, space="PSUM", tag="Ua")
                        Ub = ppU.tile([P, 256], F32, space="PSUM", tag="Ub")
                    else:
                        U1f = ppU.tile([P, 264], F32, space="PSUM", tag="Ua")
                    for t in range(cw0, cw0 + tw):
                        if t % GRP == 0:
                            do_group(t // GRP)
                        gi, sl = t // GRP, t % GRP
                        s16 = s_all[gi]
                        wv = w_all[gi][:].rearrange("p (t c) -> p t c", c=WC)
                        lhs = s16[:].rearrange("p (t c) -> p t c", c=P)[:, sl, :]
                        st = (t == cw0)
                        sp = (t == cw0 + tw - 1)
                        if layer == 2:
                            nc.tensor.matmul(out=Ua[:], lhsT=lhs,
                                             rhs=wv[:, sl, 0:264],
                                             start=st, stop=sp)
                            nc.tensor.matmul(out=Ub[:], lhsT=lhs,
                                             rhs=wv[:, sl, 264:520],
                                             start=st, stop=sp)
                        else:
                            nc.tensor.matmul(out=U1f[:, 0:72], lhsT=lhs,
                                             rhs=wv[:, sl, 0:72],
                                             start=st, stop=sp)
                    n0 = w * P
                    if layer == 1:
                        dse = ptail.tile([P, 8], F32, tag="dse")
                        nc.vector.tensor_scalar_add(out=dse[:], in0=U1f[:, 64:72],
                                                    scalar1=1e-30)
                        rd = ptail.tile([P, 8], F32, tag="rd")
                        nc.vector.reciprocal(out=rd[:], in_=dse[:])
                        h2a = ptail.tile([P, D1], F32, tag="h2a")
                        nc.vector.tensor_tensor(
                            out=h2a[:].rearrange("p (h f) -> p h f", h=8),
                            in0=U1f[:, 0:64].rearrange("p (h f) -> p h f", h=8),
                            in1=rd[:].to_broadcast([P, 8, 8]),
                            op=mybir.AluOpType.mult)
                        nc.vector.tensor_tensor(out=h2a[:], in0=h2a[:], in1=b1rep[:],
                                                op=mybir.AluOpType.add)
                        ex = ptail.tile([P, D1], F32, tag="ex")
                        nc.scalar.activation(ex[:], h2a[:],
                                             mybir.ActivationFunctionType.Exp)
                        exm = ptail.tile([P, D1], F32, tag="exm")
                        nc.vector.tensor_scalar(out=exm[:], in0=ex[:], scalar1=1.0,
                                                scalar2=-1.0, op0=mybir.AluOpType.min,
                                                op1=mybir.AluOpType.add)
                        rl = ptail.tile([P, D1], F32, tag="rl")
                        nc.vector.tensor_scalar_max(out=rl[:], in0=h2a[:], scalar1=0.0)
                        h2eb = ptail.tile([P, D1], BF16, tag="h2eb")
                        nc.vector.tensor_tensor(out=h2eb[:], in0=exm[:], in1=rl[:],
                                                op=mybir.AluOpType.add)
                        nc.sync.dma_start(
                            out=t2loc[n0:n0 + P, 0:D1].bitcast(BF16), in_=h2eb[:])
                        aa = ptail.tile([P, 1024], BF16, tag="aa")
                        nc.vector.tensor_tensor(
                            out=aa[:].rearrange("p (k f) -> p k f", k=16),
                            in0=mid_bcast(h2eb[:], 16),
                            in1=AArep[:].rearrange("p (k f) -> p k f", k=16),
                            op=mybir.AluOpType.mult)
                        asad = ptail.tile([P, 16], F32, tag="asad")
                        nc.vector.tensor_reduce(
                            out=asad[:],
                            in_=aa[:].rearrange("p (k f) -> p k f", k=16),
                            axis=mybir.AxisListType.X, op=mybir.AluOpType.add)
                        nc.sync.dma_start(
                            out=t2loc[n0:n0 + P, D1:TCOL].bitcast(F32),
                            in_=asad[:, 0:8])
                        nc.vector.tensor_copy(out=ad2w[:, w * 16:w * 16 + 8],
                                              in_=asad[:, 8:16])
                        nc.vector.tensor_tensor(out=ad2w[:, w * 16 + 8:w * 16 + 16],
                                                in0=asad[:, 8:16],
                                                in1=ad2w[:, w * 16:w * 16 + 8],
                                                op=mybir.AluOpType.subtract)
                    else:
                        dse = ptail.tile([P, 8], F32, tag="dse")
                        nc.vector.tensor_scalar_add(out=dse[:], in0=Ua[:, 256:264],
                                                    scalar1=1e-30)
                        rd = ptail.tile([P, 8], F32, tag="rd")
                        nc.vector.reciprocal(out=rd[:], in_=dse[:])
                        Un = ptail.tile([P, 512], F32, tag="Un")
                        nc.vector.tensor_tensor(
                            out=Un[:, 0:256].rearrange("p (h f) -> p h f", h=4),
                            in0=Ua[:, 0:256].rearrange("p (h f) -> p h f", h=4),
                            in1=rd[:, 0:4].to_broadcast([P, 4, F2]),
                            op=mybir.AluOpType.mult)
                        nc.vector.tensor_tensor(
                            out=Un[:, 256:512].rearrange("p (h f) -> p h f", h=4),
                            in0=Ub[:].rearrange("p (h f) -> p h f", h=4),
                            in1=rd[:, 4:8].to_broadcast([P, 4, F2]),
                            op=mybir.AluOpType.mult)
                        YT = pp2.tile([F2, P], F32, space="PSUM", tag="YT")
                        for ci in range(4):
                            T_ps = pp2.tile([P, P], F32, space="PSUM", tag="Tps")
                            nc.tensor.matmul(out=T_ps[:],
                                             lhsT=Un[:, ci * P:(ci + 1) * P],
                                             rhs=I128[:], is_transpose=True,
                                             start=True, stop=True)
                            UTb = ptail.tile([P, P], BF16, tag="UTb")
                            nc.vector.tensor_copy(out=UTb[:], in_=T_ps[:])
                            nc.tensor.matmul(out=YT[:], lhsT=w2cb_sb[ci][:],
                                             rhs=UTb[:], start=(ci == 0),
                                             stop=(ci == 3))
                        Yb = ptail.tile([F2, P], F32, tag="Yb")
                        nc.scalar.activation(Yb[:], YT[:],
                                             mybir.ActivationFunctionType.Identity,
                                             bias=b2col[:], scale=1.0)
                        nc.sync.dma_start(out=outT_d[:, n0:n0 + P], in_=Yb[:])

            edge_phase(t1full, ad1w, 1)
            nc.gpsimd.collective_compute(
                "AllGather", mybir.AluOpType.bypass,
                replica_groups=[list(range(NC))],
                ins=[t2loc[:, :].opt()],
                outs=[t2full[0:NC * SPAD, :].opt()],
            )
            edge_phase(t2full, ad2w, 2)

    nc.compile()
    return nc


def kernel(**inputs):
    global LAST_EXEC_NS
    _register_profile_hook()
    from concourse import bass_utils

    in_maps, meta = host_prep(inputs)
    nc = build(meta)
    trace = os.environ.get("GAT_TRACE", "1") == "1"
    try:
        res = bass_utils.run_bass_kernel_spmd(
            nc, in_maps, core_ids=list(range(NC)), trace=trace)
    except Exception:
        if not trace:
            raise
        res = bass_utils.run_bass_kernel_spmd(
            nc, in_maps, core_ids=list(range(NC)), trace=False)
    LAST_EXEC_NS = res.exec_time_ns
    SLICE = meta["SLICE"]
    out = np.empty((N, F2), np.float32)
    for c in range(NC):
        out[c * SLICE:(c + 1) * SLICE] = np.ascontiguousarray(
            res.results[c]["outT"].T[:SLICE])
    return out
